# revision 22
# baseline (speedup 1.0000x reference)
"""Trainium2 Bass kernel for AdvancedGNNPredictorV2 (2-layer GAT + BN + mean-pool + MLP).

Contract: kernel(**inputs) takes FULL numpy inputs, returns FULL [512, 2] output.
Internally: nodes (and incident edges, partitioned by destination) are sharded
across 8 NeuronCores; small weights replicated; gather tables all-gathered
(chunked, overlapped with the prelude GEMM); BN stats and pooled graph sums
all-reduced.

Self-contained: all shapes hardcoded for N=50000, E=800000, D_IN=128, H=4, C=64, G=512.
"""
import math
import numpy as np

from concourse import bass, bacc, mybir, tile
from concourse.bass_utils import run_bass_kernel_spmd

# Problem constants
N = 50000
E = 800000
D_IN = 128
H = 4
C = 64
HC = 256
G = 512
EPS = 1e-5
SLOPE = 0.2

M = 8                     # cores
NCN = N // M              # 6250 nodes per core
P = 128
NW = math.ceil(NCN / P)   # 49 node windows per core
LAST_ROWS = NCN - (NW - 1) * P  # 106
TW = HC + H               # 260 table row width (h || es)

F32 = mybir.dt.float32
F16 = mybir.dt.float16
I32 = mybir.dt.int32
I16 = mybir.dt.int16

TBL_DT = mybir.dt.float8e4  # gather table dtype (e4m3)
TWP = 512         # padded table row elems (512B at fp8, 256B-multiple for dma_gather)
T_SPLIT = 25000   # rows < T_SPLIT use the low gather view
HI_BASE = N - 32768  # 17232; high gather reads tfull[HI_BASE:], idx = row - HI_BASE

# AllGather chunking: windows per chunk (sum = NW)
CH_WINDOWS = [10, 10, 10, 10, 9]
NCH = len(CH_WINDOWS)
CH_WSTART = [sum(CH_WINDOWS[:k]) for k in range(NCH)]
CH_RSTART = [ws * P for ws in CH_WSTART]                       # local row starts
CH_REND = [min((CH_WSTART[k] + CH_WINDOWS[k]) * P, NCN) for k in range(NCH)]
CH_ROWS = [CH_REND[k] - CH_RSTART[k] for k in range(NCH)]      # rows per chunk
CH_GSTART = [8 * CH_RSTART[k] for k in range(NCH)]             # global start in tfull

PFP = 2  # gather prefetch depth (window-pairs ahead)


# ---------------------------------------------------------------- host side

def _new_row(src):
    """Map global node id -> row in the chunk-reordered tfull layout."""
    r = src // NCN
    l = src % NCN
    row = np.empty_like(src)
    for k in range(NCH):
        m = (l >= CH_RSTART[k]) & (l < CH_REND[k])
        row[m] = CH_GSTART[k] + r[m] * CH_ROWS[k] + (l[m] - CH_RSTART[k])
    return row


def _host_meta(edge_index: np.ndarray, batch: np.ndarray):
    """Partition/sort edges by destination; build per-core dma_gather metadata.

    One table in chunk-reordered rank-major node order (so the AllGather can be
    chunked), one AllGather chain per layer. int16 gather indices cannot
    address all 50000 rows, so edges are split at T_SPLIT: low edges gather
    from tfull[0:], high edges from the tfull[HI_BASE:] view. Per dst-window,
    each group is padded to a chunk multiple (sentinels: row 0, drel=300).
    """
    src = edge_index[0].astype(np.int64)
    dst = edge_index[1].astype(np.int64)
    owner = dst // NCN

    row = _new_row(src)
    in_a = row < T_SPLIT
    srow = np.where(in_a, row, row - HI_BASE)

    percore = []
    na_max = nb_max = 1
    for c in range(M):
        m = owner == c
        sr, sa = srow[m], in_a[m]
        dl = (dst[m] - c * NCN).astype(np.int64)
        order = np.argsort(dl, kind="stable")
        sr, sa, dl = sr[order], sa[order], dl[order]
        w = dl // P
        aa, bb = [], []
        for wi in range(NW):
            sel = w == wi
            srw, saw, dlw = sr[sel], sa[sel], dl[sel]
            aa.append((srw[saw], dlw[saw]))
            bb.append((srw[~saw], dlw[~saw]))
            na_max = max(na_max, math.ceil(saw.sum() / P))
            nb_max = max(nb_max, math.ceil((~saw).sum() / P))
        percore.append((aa, bb))
    NLOW, NHIGH = int(na_max), int(nb_max)
    K2 = NLOW + NHIGH

    def wrap16(a):
        return np.tile(a.reshape(-1, 16).T, (8, 1)).astype(np.int16)

    metas = []
    for c in range(M):
        aa, bb = percore[c]
        lowidx = np.zeros((P, NW * NLOW * 8), np.int16)
        highidx = np.zeros((P, NW * NHIGH * 8), np.int16)
        drel = np.full((P, NW * K2), -1, np.int8)
        drelT = np.full((NW, K2 * P), -1, np.int8)
        for wi in range(NW):
            sa_, da_ = aa[wi]
            sb_, db_ = bb[wi]
            na, nb = len(sa_), len(sb_)
            sap = np.zeros(NLOW * P, np.int64); sap[:na] = sa_
            sbp = np.zeros(NHIGH * P, np.int64); sbp[:nb] = sb_
            lowidx[:, wi * NLOW * 8 : (wi + 1) * NLOW * 8] = wrap16(sap)
            highidx[:, wi * NHIGH * 8 : (wi + 1) * NHIGH * 8] = wrap16(sbp)
            dr = np.full(K2 * P, -1, np.int64)
            dr[:na] = da_ - wi * P
            dr[NLOW * P : NLOW * P + nb] = db_ - wi * P
            drel[:, wi * K2 : (wi + 1) * K2] = dr.reshape(K2, P).T.astype(np.int8)
            drelT[wi, :] = dr.astype(np.int8)

        b_c = batch[c * NCN : (c + 1) * NCN].astype(np.int64)
        g_base = int(b_c[0])
        gr = (b_c - g_base).astype(np.float32)
        assert gr.max() < P, "more than 128 graphs per core not supported"
        gr_pad = np.full(NW * P, 500.0, np.float32)
        gr_pad[:NCN] = gr
        grel = gr_pad.reshape(NW, P).T.copy()
        scatidx = np.minimum(g_base + np.arange(P), G).astype(np.int32)[:, None]

        metas.append(
            dict(
                lowidx=np.ascontiguousarray(lowidx),
                highidx=np.ascontiguousarray(highidx),
                drel=np.ascontiguousarray(drel),
                drelT=np.ascontiguousarray(drelT),
                grel=np.ascontiguousarray(grel),
                scatidx=scatidx,
            )
        )
    return metas, (NLOW, NHIGH)


def _att_mat(att):
    """[H, C] attention vector -> block-diagonal [HC, H] matrix."""
    m = np.zeros((HC, H), np.float32)
    a = np.asarray(att, np.float32).reshape(H, C)
    for h in range(H):
        m[h * C : (h + 1) * C, h] = a[h]
    return m


# ---------------------------------------------------------------- device program

def _build_program(split: tuple, dbg: bool = False):
    NLOW, NHIGH = split
    K_w = NLOW + NHIGH
    nc = bacc.Bacc("TRN2", target_bir_lowering=False, debug=False, num_devices=M,
                   num_swdge_queues=4, dynamic_dma_scratch_size=45056)
    RG = [list(range(M))]
    ECOLS = NW * K_w

    # I/O
    xcT = nc.dram_tensor("xcT", [D_IN, NCN], F16, kind="ExternalInput")
    wcat1 = nc.dram_tensor("wcat1", [D_IN, 2 * HC], F16, kind="ExternalInput")
    wse1 = nc.dram_tensor("wse1", [D_IN, 2 * H], F16, kind="ExternalInput")
    wcat2 = nc.dram_tensor("wcat2", [HC, 2 * HC], F16, kind="ExternalInput")
    wse2 = nc.dram_tensor("wse2", [HC, 2 * H], F16, kind="ExternalInput")
    biasr = [nc.dram_tensor(f"bias{l}", [P, HC], F32, kind="ExternalInput") for l in (1, 2)]
    bngc = nc.dram_tensor("bngc", [P, 2], F32, kind="ExternalInput")   # bn1 gamma cols
    bnbc = nc.dram_tensor("bnbc", [P, 2], F32, kind="ExternalInput")   # bn1 beta cols
    bng2 = nc.dram_tensor("bng2", [1, HC], F32, kind="ExternalInput")
    bnb2 = nc.dram_tensor("bnb2", [1, HC], F32, kind="ExternalInput")
    fc1w = nc.dram_tensor("fc1w", [HC, 128], F32, kind="ExternalInput")
    fc2w = nc.dram_tensor("fc2w", [128, 2], F32, kind="ExternalInput")
    fc1b = nc.dram_tensor("fc1b", [P, 128], F32, kind="ExternalInput")
    fc2b = nc.dram_tensor("fc2b", [P, 2], F32, kind="ExternalInput")
    rcnt = nc.dram_tensor("rcnt", [G, 1], F32, kind="ExternalInput")
    grel_d = nc.dram_tensor("grel", [P, NW], F32, kind="ExternalInput")
    scatidx_d = nc.dram_tensor("scatidx", [P, 1], I32, kind="ExternalInput")
    lowidx_d = nc.dram_tensor("lowidx", [P, NW * NLOW * 8], I16, kind="ExternalInput")
    highidx_d = nc.dram_tensor("highidx", [P, NW * NHIGH * 8], I16, kind="ExternalInput")
    drel_d = nc.dram_tensor("drel", [P, ECOLS], mybir.dt.int8, kind="ExternalInput")
    drelT_d = nc.dram_tensor("drelT", [NW, K_w * P], mybir.dt.int8, kind="ExternalInput")
    out_d = nc.dram_tensor("out", [G, 2], F32, kind="ExternalOutput")

    # Internal DRAM
    tshard_c = [
        [nc.dram_tensor(f"tshard{l}_{k}", [CH_ROWS[k], TWP], TBL_DT, kind="Internal")
         for k in range(NCH)]
        for l in (1, 2)
    ]
    tfull = [
        nc.dram_tensor(f"tfull{l}", [N, TWP], TBL_DT, kind="Internal", addr_space="Shared")
        for l in (1, 2)
    ]
    stats_in = [nc.dram_tensor(f"statsin{l}", [2 * HC, 1], F32, kind="Internal") for l in (1, 2)]
    stats_out = [
        nc.dram_tensor(f"statsout{l}", [2 * HC, 1], F32, kind="Internal", addr_space="Shared")
        for l in (1, 2)
    ]
    pooled_in = nc.dram_tensor("pooledin", [G + 1, HC], F32, kind="Internal")
    pooled_out = nc.dram_tensor("pooledout", [G, HC], F32, kind="Internal", addr_space="Shared")

    if dbg:
        dbg_se = nc.dram_tensor("dbg_se", [P, NW, 2 * H], F32, kind="ExternalOutput")
        dbg_tfull = nc.dram_tensor("dbg_tfull", [N, TWP], TBL_DT, kind="ExternalOutput")
        dbg_s = nc.dram_tensor("dbg_s", [P, NLOW + NHIGH, H], F32, kind="ExternalOutput")
        dbg_ex = nc.dram_tensor("dbg_ex", [P, NLOW + NHIGH, H], F32, kind="ExternalOutput")
        dbg_agg = nc.dram_tensor("dbg_agg", [P, TW], F32, kind="ExternalOutput")
        dbg_hpre = nc.dram_tensor("dbg_hpre", [P, NW, HC], F32, kind="ExternalOutput")
        dbg_stats = nc.dram_tensor("dbg_stats", [2 * HC, 1], F32, kind="ExternalOutput")
        dbg_bn = nc.dram_tensor("dbg_bn", [P, 4], F32, kind="ExternalOutput")
        dbg_hpre2 = nc.dram_tensor("dbg_hpre2", [P, NW, HC], F32, kind="ExternalOutput")
        dbg_hbn2 = nc.dram_tensor("dbg_hbn2", [P, NW, HC], F32, kind="ExternalOutput")
        dbg_pool = nc.dram_tensor("dbg_pool", [G, HC], F32, kind="ExternalOutput")
        dbg_se2 = nc.dram_tensor("dbg_se2", [P, NW, 2 * H], F32, kind="ExternalOutput")
        dbg_stats2 = nc.dram_tensor("dbg_stats2", [2 * HC, 1], F32, kind="ExternalOutput")
        dbg_scale2 = nc.dram_tensor("dbg_scale2", [P, 2 * HC], F32, kind="ExternalOutput")

    from concourse.masks import make_identity

    with tile.TileContext(nc) as tc:
        with (
            tc.tile_pool(name="persist", bufs=1) as pp,
            tc.tile_pool(name="const", bufs=1) as cp,
            tc.tile_pool(name="work", bufs=2) as wp,
            tc.tile_pool(name="edge", bufs=2) as ep,
            tc.tile_pool(name="gbig", bufs=3) as gbp,
            tc.tile_pool(name="psum_tr", bufs=2, space="PSUM") as ptr,
            tc.tile_pool(name="psum_mm", bufs=2, space="PSUM") as pmm,
            tc.tile_pool(name="psum_agg", bufs=2, space="PSUM") as pagg,
            tc.tile_pool(name="psum_ed", bufs=2, space="PSUM") as ped,
        ):
            # ---------------- constants
            identity = cp.tile([P, P], F32)
            make_identity(nc, identity[:])
            identity_h = cp.tile([P, P], F16)
            nc.vector.tensor_copy(identity_h[:], identity[:])
            iota_i = cp.tile([P, P], I32)
            nc.gpsimd.iota(iota_i[:], pattern=[[1, P]], base=0, channel_multiplier=0)
            iota_f = cp.tile([P, P], F32)
            nc.vector.tensor_copy(iota_f[:], iota_i[:])
            iota_h = cp.tile([P, P], F16)
            nc.vector.tensor_copy(iota_h[:], iota_i[:])
            iota_ci = cp.tile([P, 1], I32)
            nc.gpsimd.iota(iota_ci[:], pattern=[[0, 1]], base=0, channel_multiplier=1)
            iota_col = cp.tile([P, 1], F32)
            nc.vector.tensor_copy(iota_col[:], iota_ci[:])
            ones_col = cp.tile([P, 1], F16)
            nc.gpsimd.memset(ones_col[:], 1.0)

            wcat1_s = cp.tile([D_IN, 2 * HC], F16)
            nc.sync.dma_start(wcat1_s[:], wcat1[:])
            wse1_s = cp.tile([D_IN, 2 * H], F16)
            nc.sync.dma_start(wse1_s[:], wse1[:])
            wcat2_s0 = cp.tile([P, 2 * HC], F16)
            nc.sync.dma_start(wcat2_s0[:], wcat2[0:P, :])
            wcat2_s1 = cp.tile([P, 2 * HC], F16)
            nc.sync.dma_start(wcat2_s1[:], wcat2[P:HC, :])
            wse2_s0 = cp.tile([P, 2 * H], F16)
            nc.sync.dma_start(wse2_s0[:], wse2[0:P, :])
            wse2_s1 = cp.tile([P, 2 * H], F16)
            nc.sync.dma_start(wse2_s1[:], wse2[P:HC, :])
            bias_s = [cp.tile([P, HC], F32, name=f"bias_s{l}") for l in range(2)]
            for l in range(2):
                nc.sync.dma_start(bias_s[l][:], biasr[l][:])
            bngc_s = cp.tile([P, 2], F32)
            nc.sync.dma_start(bngc_s[:], bngc[:])
            bnbc_s = cp.tile([P, 2], F32)
            nc.sync.dma_start(bnbc_s[:], bnbc[:])

            # edge metadata, loaded once (shared by both layers)
            drel_sb = cp.tile([P, ECOLS], mybir.dt.int8)
            nc.sync.dma_start(drel_sb[:], drel_d[:])
            grel_sb = cp.tile([P, NW], F32)
            nc.sync.dma_start(grel_sb[:], grel_d[:])

            # persistent node buffers
            h_pre = pp.tile([P, NW, HC], F16)   # pre-BN node features
            se_sb = pp.tile([P, NW, 2 * H], F16)  # per-node es||ed (this layer)
            nc.gpsimd.memset(h_pre[:], 0.0)
            nc.gpsimd.memset(se_sb[:], 0.0)

            # BN1 column coefs (written after layer-1 stats AR)
            bn1_scale = pp.tile([P, 2], F32)
            bn1_shift = pp.tile([P, 2], F32)

            def rows_of(w):
                return LAST_ROWS if w == NW - 1 else P

            # ================= per-layer =================
            for l in range(2):
                # ---- prelude: GEMM h||res + es/ed, build tables, chunked AllGather
                for k in range(NCH):
                    for w in range(CH_WSTART[k], CH_WSTART[k] + CH_WINDOWS[k]):
                        rows = rows_of(w)
                        psum_h = pmm.tile([P, 2 * HC], F32, tag="psum_h")
                        psum_es = ped.tile([P, 2 * H], F32, tag="ped")
                        if l == 0:
                            lhs = wp.tile([P, P], F16, tag="xT")
                            nc.sync.dma_start(lhs[:, :rows], xcT[:, w * P : w * P + rows])
                            nc.tensor.matmul(
                                psum_h[:rows], lhsT=lhs[:, :rows], rhs=wcat1_s[:],
                                start=True, stop=True,
                            )
                            nc.tensor.matmul(
                                psum_es[:rows], lhsT=lhs[:, :rows], rhs=wse1_s[:],
                                start=True, stop=True,
                            )
                        else:
                            hin = h_pre[:, w, :]
                            for kk in range(2):
                                tr = ptr.tile([P, P], F16, tag="tr")
                                nc.tensor.transpose(
                                    tr[:, :rows], hin[:rows, kk * P : (kk + 1) * P],
                                    identity_h[:rows, :rows],
                                )
                                # fused BN1 apply + relu on the transposed tile
                                hT = wp.tile([P, P], F16, tag="xT")
                                nc.scalar.activation(
                                    hT[:, :rows], tr[:, :rows],
                                    mybir.ActivationFunctionType.Relu,
                                    bias=bn1_shift[:, kk : kk + 1],
                                    scale=bn1_scale[:, kk : kk + 1],
                                )
                                nc.tensor.matmul(
                                    psum_h[:rows],
                                    lhsT=hT[:, :rows],
                                    rhs=(wcat2_s0 if kk == 0 else wcat2_s1)[:],
                                    start=(kk == 0),
                                    stop=(kk == 1),
                                )
                                nc.tensor.matmul(
                                    psum_es[:rows],
                                    lhsT=hT[:, :rows],
                                    rhs=(wse2_s0 if kk == 0 else wse2_s1)[:],
                                    start=(kk == 0),
                                    stop=(kk == 1),
                                )
                        # residual + bias -> h_pre tile
                        nc.vector.tensor_add(
                            h_pre[:rows, w, :], psum_h[:rows, HC : 2 * HC], bias_s[l][:rows]
                        )
                        nc.scalar.copy(se_sb[:rows, w, :], psum_es[:rows])
                        # table row: h || es (fp16)
                        h_tb = wp.tile([P, TW], TBL_DT, tag="h_tb")
                        nc.scalar.copy(h_tb[:rows, 0:HC], psum_h[:rows, 0:HC])
                        nc.scalar.copy(h_tb[:rows, HC:TW], psum_es[:rows, 0:H])
                        ro = (w - CH_WSTART[k]) * P
                        nc.sync.dma_start(
                            tshard_c[l][k][ro : ro + rows, 0:TW], h_tb[:rows]
                        )
                    # fire this chunk's AllGather
                    nc.gpsimd.collective_compute(
                        "AllGather",
                        mybir.AluOpType.bypass,
                        replica_groups=RG,
                        ins=[tshard_c[l][k].ap()],
                        outs=[tfull[l][CH_GSTART[k] : CH_GSTART[k] + 8 * CH_ROWS[k], :]],
                    )

                if dbg and l == 0:
                    nc.gpsimd.dma_start(dbg_se[:], se_sb[:])
                    nc.gpsimd.dma_start(dbg_tfull[:], tfull[0][:])

                # ---- edge pass
                stats_acc = wp.tile([1, 2 * HC], F32, tag="stats_acc")
                nc.gpsimd.memset(stats_acc[:], 0.0)
                gtiles = {}

                NPAIR = (NW + 1) // 2

                def emit_gathers(p):
                    w0 = 2 * p
                    nwin = min(2, NW - w0)
                    li = ep.tile([P, nwin * NLOW * 8], I16, tag="li", bufs=3)
                    nc.sync.dma_start(li[:], lowidx_d[:, w0 * NLOW * 8 : (w0 + nwin) * NLOW * 8])
                    hi = ep.tile([P, nwin * NHIGH * 8], I16, tag="hi", bufs=3)
                    nc.sync.dma_start(hi[:], highidx_d[:, w0 * NHIGH * 8 : (w0 + nwin) * NHIGH * 8])
                    ga = gbp.tile([P, nwin * NLOW, TWP], TBL_DT, tag="ga")
                    nc.gpsimd.dma_gather(
                        out_ap=ga[:],
                        in_ap=tfull[l][0:T_SPLIT, :],
                        idxs_ap=li[:],
                        num_idxs=nwin * NLOW * P,
                        num_idxs_reg=nwin * NLOW * P,
                        elem_size=TWP,
                        single_packet=False,
                        queue_num=2 * (p % 2),
                    )
                    gb = gbp.tile([P, nwin * NHIGH, TWP], TBL_DT, tag="gb")
                    nc.gpsimd.dma_gather(
                        out_ap=gb[:],
                        in_ap=tfull[l][HI_BASE:N, :],
                        idxs_ap=hi[:],
                        num_idxs=nwin * NHIGH * P,
                        num_idxs_reg=nwin * NHIGH * P,
                        elem_size=TWP,
                        single_packet=False,
                        queue_num=2 * (p % 2) + 1,
                    )
                    gtiles[p] = (ga, gb)

                for p in range(min(PFP + 1, NPAIR)):
                    emit_gathers(p)

                for w in range(NW):
                    rows = rows_of(w)
                    pair = w // 2
                    gpa, gpb = gtiles[pair]
                    wo = w - 2 * pair
                    ga = gpa[:, wo * NLOW : (wo + 1) * NLOW, :]
                    gb = gpb[:, wo * NHIGH : (wo + 1) * NHIGH, :]
                    if w == 2 * pair and pair + PFP + 1 < NPAIR:
                        emit_gathers(pair + PFP + 1)
                    if w == 2 * pair + 1 or w == NW - 1:
                        gtiles.pop(pair)
                    # ind_T via int8 broadcast DMA (half the bytes of the f16 version)
                    drelT_rep = ep.tile([P, K_w * P], mybir.dt.int8, tag="drelT_rep")
                    nc.sync.dma_start(
                        drelT_rep[:], drelT_d[w : w + 1, :].to_broadcast([P, K_w * P])
                    )
                    ind_T = ep.tile([P, K_w, P], F16, tag="indT")
                    nc.vector.tensor_scalar(
                        out=ind_T[:],
                        in0=drelT_rep[:].rearrange("d (t e) -> d t e", e=P),
                        scalar1=iota_col[:, 0:1],
                        scalar2=None,
                        op0=mybir.AluOpType.is_equal,
                    )
                    # ed broadcast node->edge via transposed indicator matmuls
                    psum_ed = ped.tile([P, K_w * H], F32, tag="ped")
                    for t in range(K_w):
                        nc.tensor.matmul(
                            psum_ed[:, t * H : (t + 1) * H],
                            lhsT=ind_T[:, t, :],
                            rhs=se_sb[:, w, H : 2 * H],
                            start=True,
                            stop=True,
                        )
                    # s = es[src] + ed[dst]; scores on scalar engine
                    s_t = wp.tile([P, K_w, H], F16, tag="s_t")
                    nc.vector.tensor_tensor(
                        out=s_t[:, 0:NLOW, :],
                        in0=psum_ed[:].rearrange("p (t h) -> p t h", h=H)[:, 0:NLOW, :],
                        in1=ga[:, :, HC:TW],
                        op=mybir.AluOpType.add,
                    )
                    nc.vector.tensor_tensor(
                        out=s_t[:, NLOW:K_w, :],
                        in0=psum_ed[:].rearrange("p (t h) -> p t h", h=H)[:, NLOW:K_w, :],
                        in1=gb[:, :, HC:TW],
                        op=mybir.AluOpType.add,
                    )
                    lr_t = wp.tile([P, K_w, H], F16, tag="lr_t")
                    nc.vector.scalar_tensor_tensor(
                        out=lr_t[:], in0=s_t[:], scalar=SLOPE, in1=s_t[:],
                        op0=mybir.AluOpType.mult, op1=mybir.AluOpType.max,
                    )
                    ex_t = wp.tile([P, K_w, H], F16, tag="ex_t")
                    nc.scalar.activation(ex_t[:], lr_t[:], mybir.ActivationFunctionType.Exp)
                    if dbg and l == 0 and w == 0:
                        nc.gpsimd.dma_start(dbg_s[:], s_t[:])
                        nc.gpsimd.dma_start(dbg_ex[:], ex_t[:])
                    # indicator + staging per half
                    inds = []
                    stgs = []
                    for (tg, lo, nch), g_h in zip(
                        (("a", 0, NLOW), ("b", NLOW, NHIGH)), (ga, gb)
                    ):
                        sl = slice(lo, lo + nch)
                        ind_h = ep.tile([P, nch, P], F16, tag=f"ind_{tg}")
                        nc.vector.tensor_tensor(
                            out=ind_h[:],
                            in0=drel_sb[:, w * K_w + lo : w * K_w + lo + nch, None].to_broadcast(
                                [P, nch, P]
                            ),
                            in1=iota_h[:, None, :].to_broadcast([P, nch, P]),
                            op=mybir.AluOpType.is_equal,
                        )
                        inds.append(ind_h)
                        stg_h = ep.tile([P, nch, TW], F16, tag=f"stg_{tg}")
                        nc.vector.tensor_tensor(
                            out=stg_h[:, :, 0:HC].rearrange("p t (h c) -> p t h c", h=H),
                            in0=g_h[:, :, 0:HC].rearrange("p t (h c) -> p t h c", h=H),
                            in1=ex_t[:, sl, :, None].to_broadcast([P, nch, H, C]),
                            op=mybir.AluOpType.mult,
                        )
                        nc.scalar.copy(stg_h[:, :, HC:TW], ex_t[:, sl, :])
                        stgs.append(stg_h)
                    # aggregate
                    agg = pagg.tile([P, TW], F32, tag="agg")
                    for t in range(K_w):
                        half = 0 if t < NLOW else 1
                        tl = t if t < NLOW else t - NLOW
                        nc.tensor.matmul(
                            agg[:],
                            lhsT=inds[half][:, tl, :],
                            rhs=stgs[half][:, tl, :],
                            start=(t == 0),
                            stop=(t == K_w - 1),
                        )
                    if dbg and l == 0 and w == 0:
                        aggc = wp.tile([P, TW], F32, tag="aggc")
                        nc.scalar.copy(aggc[:], agg[:])
                        nc.sync.dma_start(dbg_agg[:], aggc[:])
                    # evict: h_pre += agg[:, :256] * rden
                    rden = wp.tile([P, H], F32, tag="rden")
                    nc.vector.reciprocal(rden[:rows], agg[:rows, HC:TW])
                    agn = wp.tile([P, HC], F32, tag="agn")
                    nc.vector.tensor_tensor(
                        out=agn[:rows].rearrange("p (h c) -> p h c", h=H),
                        in0=agg[:rows, 0:HC].rearrange("p (h c) -> p h c", h=H),
                        in1=rden[:rows, :, None].to_broadcast([rows, H, C]),
                        op=mybir.AluOpType.mult,
                    )
                    nc.vector.tensor_add(h_pre[:rows, w, :], agn[:rows], h_pre[:rows, w, :])
                    # BN stats: per-window matmuls + SBUF accumulate
                    hsq = wp.tile([P, HC], F16, tag="hsq")
                    nc.scalar.square(hsq[:rows], h_pre[:rows, w, :])
                    st = ptr.tile([1, 2 * HC], F32, tag="tr")
                    nc.tensor.matmul(
                        st[0:1, 0:HC], lhsT=ones_col[:rows, :], rhs=h_pre[:rows, w, :],
                        start=True, stop=True,
                    )
                    nc.tensor.matmul(
                        st[0:1, HC : 2 * HC], lhsT=ones_col[:rows, :], rhs=hsq[:rows],
                        start=True, stop=True,
                    )
                    nc.vector.tensor_add(stats_acc[:], stats_acc[:], st[:])

                # ---- BN stats all-reduce
                nc.sync.dma_start(stats_in[l].ap().rearrange("(a b) c -> b (a c)", b=1), stats_acc[:])
                nc.gpsimd.collective_compute(
                    "AllReduce",
                    mybir.AluOpType.add,
                    replica_groups=RG,
                    ins=[stats_in[l].ap()],
                    outs=[stats_out[l].ap()],
                )
                if l == 0:
                    # column-form coefs for the fused layer-2 prelude BN
                    sums = wp.tile([P, 2], F32, tag="sums")
                    sqs = wp.tile([P, 2], F32, tag="sqs")
                    for kk in range(2):
                        nc.sync.dma_start(
                            sums[:, kk : kk + 1], stats_out[0][kk * P : (kk + 1) * P, :]
                        )
                        nc.sync.dma_start(
                            sqs[:, kk : kk + 1], stats_out[0][HC + kk * P : HC + (kk + 1) * P, :]
                        )
                    mean_c = wp.tile([P, 2], F32, tag="mean_c")
                    nc.vector.tensor_scalar_mul(mean_c[:], sums[:], 1.0 / N)
                    var_c = wp.tile([P, 2], F32, tag="var_c")
                    nc.vector.tensor_scalar_mul(var_c[:], sqs[:], 1.0 / N)
                    msq_c = wp.tile([P, 2], F32, tag="msq_c")
                    nc.vector.tensor_tensor(out=msq_c[:], in0=mean_c[:], in1=mean_c[:], op=mybir.AluOpType.mult)
                    nc.vector.tensor_tensor(out=var_c[:], in0=var_c[:], in1=msq_c[:], op=mybir.AluOpType.subtract)
                    nc.vector.tensor_scalar_add(var_c[:], var_c[:], EPS)
                    sq_c = wp.tile([P, 2], F32, tag="sq_c")
                    nc.scalar.activation(sq_c[:], var_c[:], mybir.ActivationFunctionType.Sqrt)
                    inv_c = wp.tile([P, 2], F32, tag="inv_c")
                    nc.vector.reciprocal(inv_c[:], sq_c[:])
                    nc.vector.tensor_tensor(out=bn1_scale[:], in0=inv_c[:], in1=bngc_s[:], op=mybir.AluOpType.mult)
                    sh_c = wp.tile([P, 2], F32, tag="sh_c")
                    nc.vector.tensor_tensor(out=sh_c[:], in0=mean_c[:], in1=bn1_scale[:], op=mybir.AluOpType.mult)
                    nc.vector.tensor_tensor(out=bn1_shift[:], in0=bnbc_s[:], in1=sh_c[:], op=mybir.AluOpType.subtract)
                    if dbg:
                        nc.gpsimd.dma_start(dbg_hpre[:], h_pre[:])
                        nc.sync.dma_start(dbg_stats[:], stats_out[0][:])
                        nc.sync.dma_start(dbg_bn[:, 0:2], bn1_scale[:])
                        nc.sync.dma_start(dbg_bn[:, 2:4], bn1_shift[:])
                else:
                    if dbg:
                        nc.gpsimd.dma_start(dbg_hpre2[:], h_pre[:])
                        nc.gpsimd.dma_start(dbg_se2[:], se_sb[:])
                    # row-form BN2 apply + relu on h_pre
                    stats_g = wp.tile([1, 2 * HC], F32, tag="stats_g")
                    nc.sync.dma_start(
                        stats_g[:], stats_out[1].ap().rearrange("(a b) c -> b (a c)", b=1)
                    )
                    bngs = wp.tile([1, HC], F32, tag="bngs")
                    nc.sync.dma_start(bngs[:], bng2[:])
                    bnbs = wp.tile([1, HC], F32, tag="bnbs")
                    nc.sync.dma_start(bnbs[:], bnb2[:])
                    mrow = wp.tile([1, HC], F32, tag="mrow")
                    nc.vector.tensor_scalar_mul(mrow[:], stats_g[0:1, 0:HC], 1.0 / N)
                    vrow = wp.tile([1, HC], F32, tag="vrow")
                    nc.vector.tensor_scalar_mul(vrow[:], stats_g[0:1, HC : 2 * HC], 1.0 / N)
                    msq = wp.tile([1, HC], F32, tag="msq")
                    nc.vector.tensor_tensor(out=msq[:], in0=mrow[:], in1=mrow[:], op=mybir.AluOpType.mult)
                    nc.vector.tensor_tensor(out=vrow[:], in0=vrow[:], in1=msq[:], op=mybir.AluOpType.subtract)
                    nc.vector.tensor_scalar_add(vrow[:], vrow[:], EPS)
                    sq = wp.tile([1, HC], F32, tag="sq")
                    nc.scalar.activation(sq[:], vrow[:], mybir.ActivationFunctionType.Sqrt)
                    inv = wp.tile([1, HC], F32, tag="inv")
                    nc.vector.reciprocal(inv[:], sq[:])
                    scale = wp.tile([1, HC], F32, tag="scale")
                    nc.vector.tensor_tensor(out=scale[:], in0=inv[:], in1=bngs[:], op=mybir.AluOpType.mult)
                    shift = wp.tile([1, HC], F32, tag="shift")
                    nc.vector.tensor_tensor(out=shift[:], in0=mrow[:], in1=scale[:], op=mybir.AluOpType.mult)
                    nc.vector.tensor_tensor(out=shift[:], in0=bnbs[:], in1=shift[:], op=mybir.AluOpType.subtract)
                    scale_rep = wp.tile([P, HC], F32, tag="scale_rep")
                    nc.gpsimd.partition_broadcast(scale_rep[:], scale[:])
                    shift_rep = wp.tile([P, HC], F32, tag="shift_rep")
                    nc.gpsimd.partition_broadcast(shift_rep[:], shift[:])
                    if dbg:
                        nc.sync.dma_start(dbg_stats2[:], stats_out[1][:])
                        nc.sync.dma_start(dbg_scale2[:, 0:HC], scale_rep[:])
                        nc.sync.dma_start(dbg_scale2[:, HC:2*HC], shift_rep[:])
                    for w in range(NW):
                        rows = rows_of(w)
                        nc.vector.tensor_tensor(
                            out=h_pre[:rows, w, :], in0=h_pre[:rows, w, :], in1=scale_rep[:rows],
                            op=mybir.AluOpType.mult,
                        )
                        nc.vector.tensor_tensor(
                            out=h_pre[:rows, w, :], in0=h_pre[:rows, w, :], in1=shift_rep[:rows],
                            op=mybir.AluOpType.add,
                        )
                        nc.scalar.activation(
                            h_pre[:rows, w, :], h_pre[:rows, w, :], mybir.ActivationFunctionType.Relu
                        )

            if dbg:
                nc.gpsimd.dma_start(dbg_hbn2[:], h_pre[:])
            # ================= pooling =================
            pool_ps = pmm.tile([P, HC], F32, tag="psum_h")
            for w in range(NW):
                rows = rows_of(w)
                gind = wp.tile([P, P], F16, tag="gind")
                nc.vector.tensor_tensor(
                    out=gind[:],
                    in0=grel_sb[:, w : w + 1].to_broadcast([P, P]),
                    in1=iota_f[:],
                    op=mybir.AluOpType.is_equal,
                )
                nc.tensor.matmul(
                    pool_ps[:],
                    lhsT=gind[:rows, :],
                    rhs=h_pre[:rows, w, :],
                    start=(w == 0),
                    stop=(w == NW - 1),
                )
            pooled_sb = wp.tile([P, HC], F32, tag="pooled_sb")
            nc.scalar.copy(pooled_sb[:], pool_ps[:])
            zt = wp.tile([P, HC], F32, tag="zt")
            nc.gpsimd.memset(zt[:], 0.0)
            for r in range(0, G, P):
                nc.sync.dma_start(pooled_in[r : r + P, :], zt[:])
            nc.sync.dma_start(pooled_in[G : G + 1, :], zt[0:1, :])
            scat = wp.tile([P, 1], I32, tag="scat")
            nc.sync.dma_start(scat[:], scatidx_d[:])
            nc.gpsimd.indirect_dma_start(
                out=pooled_in[:],
                out_offset=bass.IndirectOffsetOnAxis(ap=scat[:, :1], axis=0),
                in_=pooled_sb[:],
                in_offset=None,
            )
            nc.gpsimd.collective_compute(
                "AllReduce",
                mybir.AluOpType.add,
                replica_groups=RG,
                ins=[pooled_in[0:G, :]],
                outs=[pooled_out.ap()],
            )

            if dbg:
                nc.sync.dma_start(dbg_pool[:], pooled_out[:])
            # ================= MLP =================
            fc1w_s0 = cp.tile([P, 128], F32)
            nc.sync.dma_start(fc1w_s0[:], fc1w[0:P, :])
            fc1w_s1 = cp.tile([P, 128], F32)
            nc.sync.dma_start(fc1w_s1[:], fc1w[P:HC, :])
            fc2w_s = cp.tile([P, 2], F32)
            nc.sync.dma_start(fc2w_s[:], fc2w[:])
            fc1b_s = cp.tile([P, 128], F32)
            nc.sync.dma_start(fc1b_s[:], fc1b[:])
            fc2b_s = cp.tile([P, 2], F32)
            nc.sync.dma_start(fc2b_s[:], fc2b[:])

            for tgi in range(G // P):
                pt = wp.tile([P, HC], F32, tag="pt")
                nc.sync.dma_start(pt[:], pooled_out[tgi * P : (tgi + 1) * P, :])
                rc = wp.tile([P, 1], F32, tag="rc")
                nc.sync.dma_start(rc[:], rcnt[tgi * P : (tgi + 1) * P, :])
                nc.vector.tensor_scalar_mul(pt[:], pt[:], rc[:, 0:1])
                mm1 = pmm.tile([P, 128], F32, tag="psum_h")
                for k in range(2):
                    tr = ptr.tile([P, P], F32, tag="tr")
                    nc.tensor.transpose(tr[:], pt[:, k * P : (k + 1) * P], identity[:])
                    ptT = wp.tile([P, P], F32, tag="ptT")
                    nc.scalar.copy(ptT[:], tr[:])
                    nc.tensor.matmul(
                        mm1[:],
                        lhsT=ptT[:],
                        rhs=(fc1w_s0 if k == 0 else fc1w_s1)[:],
                        start=(k == 0),
                        stop=(k == 1),
                    )
                o1 = wp.tile([P, 128], F32, tag="o1")
                nc.vector.tensor_add(o1[:], mm1[:], fc1b_s[:])
                nc.scalar.activation(o1[:], o1[:], mybir.ActivationFunctionType.Relu)
                tr2 = ptr.tile([P, P], F32, tag="tr")
                nc.tensor.transpose(tr2[:], o1[:], identity[:])
                o1T = wp.tile([P, P], F32, tag="o1T")
                nc.scalar.copy(o1T[:], tr2[:])
                mm2 = pagg.tile([P, 2], F32, tag="agg")
                nc.tensor.matmul(mm2[:], lhsT=o1T[:], rhs=fc2w_s[:], start=True, stop=True)
                oo = wp.tile([P, 2], F32, tag="oo")
                nc.vector.tensor_add(oo[:], mm2[:], fc2b_s[:])
                nc.sync.dma_start(out_d[tgi * P : (tgi + 1) * P, :], oo[:])

    nc.compile()
    return nc


_PROGRAM_CACHE: dict[tuple, object] = {}


def _get_program(split: tuple):
    if split not in _PROGRAM_CACHE:
        _PROGRAM_CACHE[split] = _build_program(split)
    return _PROGRAM_CACHE[split]


# ---------------------------------------------------------------- entry point

def kernel(
    x, edge_index, edge_attr, batch,
    W1, att_src1, att_dst1, b1, res1_W, res1_b, bn1_g, bn1_b,
    W2, att_src2, att_dst2, b2, res2_W, res2_b, bn2_g, bn2_b,
    fc1_W, fc1_b, fc2_W, fc2_b,
    _run_opts: dict | None = None,
):
    x = np.asarray(x, np.float32)
    edge_index = np.asarray(edge_index)
    batch = np.asarray(batch)

    metas, split = _host_meta(edge_index, batch)
    nc = _get_program(split)

    W1n = np.asarray(W1, np.float32)
    W2n = np.asarray(W2, np.float32)
    wcat1 = np.ascontiguousarray(np.concatenate([W1n, np.asarray(res1_W)], axis=1)).astype(np.float16)
    wcat2 = np.ascontiguousarray(np.concatenate([W2n, np.asarray(res2_W)], axis=1)).astype(np.float16)
    wse1 = np.ascontiguousarray(
        np.concatenate([W1n @ _att_mat(att_src1), W1n @ _att_mat(att_dst1)], axis=1)
    ).astype(np.float16)
    wse2 = np.ascontiguousarray(
        np.concatenate([W2n @ _att_mat(att_src2), W2n @ _att_mat(att_dst2)], axis=1)
    ).astype(np.float16)
    rep = lambda v: np.ascontiguousarray(np.tile(np.asarray(v, np.float32).reshape(1, -1), (P, 1)))
    cnt = np.bincount(batch, minlength=G).astype(np.float32)
    rcnt = (1.0 / np.maximum(cnt, 1.0)).astype(np.float32)[:, None]

    shared = dict(
        wcat1=wcat1, wse1=wse1, wcat2=wcat2, wse2=wse2,
        bias1=rep(np.asarray(b1) + np.asarray(res1_b)), bias2=rep(np.asarray(b2) + np.asarray(res2_b)),
        bngc=np.ascontiguousarray(np.asarray(bn1_g, np.float32).reshape(2, P).T),
        bnbc=np.ascontiguousarray(np.asarray(bn1_b, np.float32).reshape(2, P).T),
        bng2=np.asarray(bn2_g, np.float32).reshape(1, -1), bnb2=np.asarray(bn2_b, np.float32).reshape(1, -1),
        fc1w=np.asarray(fc1_W, np.float32), fc2w=np.asarray(fc2_W, np.float32),
        fc1b=rep(fc1_b), fc2b=rep(fc2_b),
        rcnt=rcnt,
    )
    in_maps = []
    for c in range(M):
        m = metas[c]
        in_maps.append(
            dict(
                shared,
                xcT=np.ascontiguousarray(x[c * NCN : (c + 1) * NCN].T).astype(np.float16),
                grel=m["grel"], scatidx=m["scatidx"],
                lowidx=m["lowidx"], highidx=m["highidx"], drel=m["drel"], drelT=m["drelT"],
            )
        )

    opts = _run_opts or {}
    res = run_bass_kernel_spmd(nc, in_maps, core_ids=list(range(M)), **opts)
    out = res.results[0]["out"].astype(np.float32)
    if opts:
        kernel.last_result = res  # stash for profiling harnesses
    return out


# revision 23
# speedup vs baseline: 1.2331x; 1.2331x over previous
"""Trainium2 Bass kernel for AdvancedGNNPredictorV2 (2-layer GAT + BN + mean-pool + MLP).

Contract: kernel(**inputs) takes FULL numpy inputs, returns FULL [512, 2] output.
Internally: nodes (and incident edges, partitioned by destination) are sharded
across 8 NeuronCores; small weights replicated; gather tables all-gathered
(chunked, overlapped with the prelude GEMM); BN stats and pooled graph sums
all-reduced.

Self-contained: all shapes hardcoded for N=50000, E=800000, D_IN=128, H=4, C=64, G=512.
"""
import math
import numpy as np

from concourse import bass, bacc, mybir, tile
from concourse.bass_utils import run_bass_kernel_spmd

# Problem constants
N = 50000
E = 800000
D_IN = 128
H = 4
C = 64
HC = 256
G = 512
EPS = 1e-5
SLOPE = 0.2

M = 8                     # cores
NCN = N // M              # 6250 nodes per core
P = 128
NW = math.ceil(NCN / P)   # 49 node windows per core
LAST_ROWS = NCN - (NW - 1) * P  # 106
TW = HC + H               # 260 table row width (h || es)

F32 = mybir.dt.float32
F16 = mybir.dt.float16
I32 = mybir.dt.int32
I16 = mybir.dt.int16

TBL_DT = mybir.dt.float8e4  # gather table dtype (e4m3)
TWP = 512         # padded table row elems (512B at fp8, 256B-multiple for dma_gather)
T_SPLIT = 25000   # rows < T_SPLIT use the low gather view
HI_BASE = N - 32768  # 17232; high gather reads tfull[HI_BASE:], idx = row - HI_BASE

# AllGather chunking: windows per chunk (sum = NW)
CH_WINDOWS = [10, 10, 10, 10, 9]
NCH = len(CH_WINDOWS)
CH_WSTART = [sum(CH_WINDOWS[:k]) for k in range(NCH)]
CH_RSTART = [ws * P for ws in CH_WSTART]                       # local row starts
CH_REND = [min((CH_WSTART[k] + CH_WINDOWS[k]) * P, NCN) for k in range(NCH)]
CH_ROWS = [CH_REND[k] - CH_RSTART[k] for k in range(NCH)]      # rows per chunk
CH_GSTART = [8 * CH_RSTART[k] for k in range(NCH)]             # global start in tfull

PF = 4  # gather prefetch depth (windows ahead)


# ---------------------------------------------------------------- host side

def _new_row(src):
    """Map global node id -> row in the chunk-reordered tfull layout."""
    r = src // NCN
    l = src % NCN
    row = np.empty_like(src)
    for k in range(NCH):
        m = (l >= CH_RSTART[k]) & (l < CH_REND[k])
        row[m] = CH_GSTART[k] + r[m] * CH_ROWS[k] + (l[m] - CH_RSTART[k])
    return row


def _host_meta(edge_index: np.ndarray, batch: np.ndarray):
    """Partition/sort edges by destination; build per-core dma_gather metadata.

    One table in chunk-reordered rank-major node order (so the AllGather can be
    chunked), one AllGather chain per layer. int16 gather indices cannot
    address all 50000 rows, so edges are split at T_SPLIT: low edges gather
    from tfull[0:], high edges from the tfull[HI_BASE:] view. Per dst-window,
    each group is padded to a chunk multiple (sentinels: row 0, drel=300).
    """
    src = edge_index[0].astype(np.int64)
    dst = edge_index[1].astype(np.int64)
    owner = dst // NCN

    row = _new_row(src)
    in_a = row < T_SPLIT
    srow = np.where(in_a, row, row - HI_BASE)

    percore = []
    na_max = nb_max = 1
    for c in range(M):
        m = owner == c
        sr, sa = srow[m], in_a[m]
        dl = (dst[m] - c * NCN).astype(np.int64)
        order = np.argsort(dl, kind="stable")
        sr, sa, dl = sr[order], sa[order], dl[order]
        w = dl // P
        aa, bb = [], []
        for wi in range(NW):
            sel = w == wi
            srw, saw, dlw = sr[sel], sa[sel], dl[sel]
            aa.append((srw[saw], dlw[saw]))
            bb.append((srw[~saw], dlw[~saw]))
            na_max = max(na_max, math.ceil(saw.sum() / P))
            nb_max = max(nb_max, math.ceil((~saw).sum() / P))
        percore.append((aa, bb))
    NLOW, NHIGH = int(na_max), int(nb_max)
    K2 = NLOW + NHIGH

    def wrap16(a):
        return np.tile(a.reshape(-1, 16).T, (8, 1)).astype(np.int16)

    metas = []
    for c in range(M):
        aa, bb = percore[c]
        lowidx = np.zeros((P, NW * NLOW * 8), np.int16)
        highidx = np.zeros((P, NW * NHIGH * 8), np.int16)
        drel = np.full((P, NW * K2), -1, np.int8)
        drelT = np.full((NW, K2 * P), -1, np.int8)
        for wi in range(NW):
            sa_, da_ = aa[wi]
            sb_, db_ = bb[wi]
            na, nb = len(sa_), len(sb_)
            sap = np.zeros(NLOW * P, np.int64); sap[:na] = sa_
            sbp = np.zeros(NHIGH * P, np.int64); sbp[:nb] = sb_
            lowidx[:, wi * NLOW * 8 : (wi + 1) * NLOW * 8] = wrap16(sap)
            highidx[:, wi * NHIGH * 8 : (wi + 1) * NHIGH * 8] = wrap16(sbp)
            dr = np.full(K2 * P, -1, np.int64)
            dr[:na] = da_ - wi * P
            dr[NLOW * P : NLOW * P + nb] = db_ - wi * P
            drel[:, wi * K2 : (wi + 1) * K2] = dr.reshape(K2, P).T.astype(np.int8)
            drelT[wi, :] = dr.astype(np.int8)

        b_c = batch[c * NCN : (c + 1) * NCN].astype(np.int64)
        g_base = int(b_c[0])
        gr = (b_c - g_base).astype(np.float32)
        assert gr.max() < P, "more than 128 graphs per core not supported"
        gr_pad = np.full(NW * P, 500.0, np.float32)
        gr_pad[:NCN] = gr
        grel = gr_pad.reshape(NW, P).T.copy()
        scatidx = np.minimum(g_base + np.arange(P), G).astype(np.int32)[:, None]

        metas.append(
            dict(
                lowidx=np.ascontiguousarray(lowidx),
                highidx=np.ascontiguousarray(highidx),
                drel=np.ascontiguousarray(drel),
                drelT=np.ascontiguousarray(drelT),
                grel=np.ascontiguousarray(grel),
                scatidx=scatidx,
            )
        )
    return metas, (NLOW, NHIGH)


def _att_mat(att):
    """[H, C] attention vector -> block-diagonal [HC, H] matrix."""
    m = np.zeros((HC, H), np.float32)
    a = np.asarray(att, np.float32).reshape(H, C)
    for h in range(H):
        m[h * C : (h + 1) * C, h] = a[h]
    return m


# ---------------------------------------------------------------- device program

def _build_program(split: tuple, dbg: bool = False):
    NLOW, NHIGH = split
    K_w = NLOW + NHIGH
    nc = bacc.Bacc("TRN2", target_bir_lowering=False, debug=False, num_devices=M,
                   num_swdge_queues=4, dynamic_dma_scratch_size=49152)
    RG = [list(range(M))]
    ECOLS = NW * K_w

    # I/O
    xcT = nc.dram_tensor("xcT", [D_IN, NCN], F16, kind="ExternalInput")
    wcat1 = nc.dram_tensor("wcat1", [D_IN, 2 * HC], F16, kind="ExternalInput")
    wse1 = nc.dram_tensor("wse1", [D_IN, 2 * H], F16, kind="ExternalInput")
    wcat2 = nc.dram_tensor("wcat2", [HC, 2 * HC], F16, kind="ExternalInput")
    wse2 = nc.dram_tensor("wse2", [HC, 2 * H], F16, kind="ExternalInput")
    biasr = [nc.dram_tensor(f"bias{l}", [P, HC], F32, kind="ExternalInput") for l in (1, 2)]
    bngc = nc.dram_tensor("bngc", [P, 2], F32, kind="ExternalInput")   # bn1 gamma cols
    bnbc = nc.dram_tensor("bnbc", [P, 2], F32, kind="ExternalInput")   # bn1 beta cols
    bng2 = nc.dram_tensor("bng2", [1, HC], F32, kind="ExternalInput")
    bnb2 = nc.dram_tensor("bnb2", [1, HC], F32, kind="ExternalInput")
    fc1w = nc.dram_tensor("fc1w", [HC, 128], F32, kind="ExternalInput")
    fc2w = nc.dram_tensor("fc2w", [128, 2], F32, kind="ExternalInput")
    fc1b = nc.dram_tensor("fc1b", [P, 128], F32, kind="ExternalInput")
    fc2b = nc.dram_tensor("fc2b", [P, 2], F32, kind="ExternalInput")
    rcnt = nc.dram_tensor("rcnt", [G, 1], F32, kind="ExternalInput")
    grel_d = nc.dram_tensor("grel", [P, NW], F32, kind="ExternalInput")
    scatidx_d = nc.dram_tensor("scatidx", [P, 1], I32, kind="ExternalInput")
    lowidx_d = nc.dram_tensor("lowidx", [P, NW * NLOW * 8], I16, kind="ExternalInput")
    highidx_d = nc.dram_tensor("highidx", [P, NW * NHIGH * 8], I16, kind="ExternalInput")
    drel_d = nc.dram_tensor("drel", [P, ECOLS], mybir.dt.int8, kind="ExternalInput")
    drelT_d = nc.dram_tensor("drelT", [NW, K_w * P], mybir.dt.int8, kind="ExternalInput")
    out_d = nc.dram_tensor("out", [G, 2], F32, kind="ExternalOutput")

    # Internal DRAM
    tshard_c = [
        [nc.dram_tensor(f"tshard{l}_{k}", [CH_ROWS[k], TWP], TBL_DT, kind="Internal")
         for k in range(NCH)]
        for l in (1, 2)
    ]
    tfull = [
        nc.dram_tensor(f"tfull{l}", [N, TWP], TBL_DT, kind="Internal", addr_space="Shared")
        for l in (1, 2)
    ]
    stats_in = [nc.dram_tensor(f"statsin{l}", [2 * HC, 1], F32, kind="Internal") for l in (1, 2)]
    stats_out = [
        nc.dram_tensor(f"statsout{l}", [2 * HC, 1], F32, kind="Internal", addr_space="Shared")
        for l in (1, 2)
    ]
    pooled_in = nc.dram_tensor("pooledin", [G + 1, HC], F32, kind="Internal")
    pooled_out = nc.dram_tensor("pooledout", [G, HC], F32, kind="Internal", addr_space="Shared")

    if dbg:
        dbg_se = nc.dram_tensor("dbg_se", [P, NW, 2 * H], F32, kind="ExternalOutput")
        dbg_tfull = nc.dram_tensor("dbg_tfull", [N, TWP], TBL_DT, kind="ExternalOutput")
        dbg_s = nc.dram_tensor("dbg_s", [P, NLOW + NHIGH, H], F32, kind="ExternalOutput")
        dbg_ex = nc.dram_tensor("dbg_ex", [P, NLOW + NHIGH, H], F32, kind="ExternalOutput")
        dbg_agg = nc.dram_tensor("dbg_agg", [P, TW], F32, kind="ExternalOutput")
        dbg_hpre = nc.dram_tensor("dbg_hpre", [P, NW, HC], F32, kind="ExternalOutput")
        dbg_stats = nc.dram_tensor("dbg_stats", [2 * HC, 1], F32, kind="ExternalOutput")
        dbg_bn = nc.dram_tensor("dbg_bn", [P, 4], F32, kind="ExternalOutput")
        dbg_hpre2 = nc.dram_tensor("dbg_hpre2", [P, NW, HC], F32, kind="ExternalOutput")
        dbg_hbn2 = nc.dram_tensor("dbg_hbn2", [P, NW, HC], F32, kind="ExternalOutput")
        dbg_pool = nc.dram_tensor("dbg_pool", [G, HC], F32, kind="ExternalOutput")
        dbg_se2 = nc.dram_tensor("dbg_se2", [P, NW, 2 * H], F32, kind="ExternalOutput")
        dbg_stats2 = nc.dram_tensor("dbg_stats2", [2 * HC, 1], F32, kind="ExternalOutput")
        dbg_scale2 = nc.dram_tensor("dbg_scale2", [P, 2 * HC], F32, kind="ExternalOutput")

    from concourse.masks import make_identity

    with tile.TileContext(nc) as tc:
        with (
            tc.tile_pool(name="persist", bufs=1) as pp,
            tc.tile_pool(name="const", bufs=1) as cp,
            tc.tile_pool(name="work", bufs=2) as wp,
            tc.tile_pool(name="edge", bufs=2) as ep,
            tc.tile_pool(name="gbig", bufs=5) as gbp,
            tc.tile_pool(name="psum_tr", bufs=2, space="PSUM") as ptr,
            tc.tile_pool(name="psum_mm", bufs=2, space="PSUM") as pmm,
            tc.tile_pool(name="psum_agg", bufs=2, space="PSUM") as pagg,
            tc.tile_pool(name="psum_ed", bufs=2, space="PSUM") as ped,
        ):
            # ---------------- constants
            identity = cp.tile([P, P], F32)
            make_identity(nc, identity[:])
            identity_h = cp.tile([P, P], F16)
            nc.vector.tensor_copy(identity_h[:], identity[:])
            iota_i = cp.tile([P, P], I32)
            nc.gpsimd.iota(iota_i[:], pattern=[[1, P]], base=0, channel_multiplier=0)
            iota_f = cp.tile([P, P], F32)
            nc.vector.tensor_copy(iota_f[:], iota_i[:])
            iota_h = cp.tile([P, P], F16)
            nc.vector.tensor_copy(iota_h[:], iota_i[:])
            iota_ci = cp.tile([P, 1], I32)
            nc.gpsimd.iota(iota_ci[:], pattern=[[0, 1]], base=0, channel_multiplier=1)
            iota_col = cp.tile([P, 1], F32)
            nc.vector.tensor_copy(iota_col[:], iota_ci[:])
            ones_col = cp.tile([P, 1], F16)
            nc.gpsimd.memset(ones_col[:], 1.0)

            wcat1_s = cp.tile([D_IN, 2 * HC], F16)
            nc.sync.dma_start(wcat1_s[:], wcat1[:])
            wse1_s = cp.tile([D_IN, 2 * H], F16)
            nc.sync.dma_start(wse1_s[:], wse1[:])
            wcat2_s0 = cp.tile([P, 2 * HC], F16)
            nc.sync.dma_start(wcat2_s0[:], wcat2[0:P, :])
            wcat2_s1 = cp.tile([P, 2 * HC], F16)
            nc.sync.dma_start(wcat2_s1[:], wcat2[P:HC, :])
            wse2_s0 = cp.tile([P, 2 * H], F16)
            nc.sync.dma_start(wse2_s0[:], wse2[0:P, :])
            wse2_s1 = cp.tile([P, 2 * H], F16)
            nc.sync.dma_start(wse2_s1[:], wse2[P:HC, :])
            bias_s = [cp.tile([P, HC], F32, name=f"bias_s{l}") for l in range(2)]
            for l in range(2):
                nc.sync.dma_start(bias_s[l][:], biasr[l][:])
            bngc_s = cp.tile([P, 2], F32)
            nc.sync.dma_start(bngc_s[:], bngc[:])
            bnbc_s = cp.tile([P, 2], F32)
            nc.sync.dma_start(bnbc_s[:], bnbc[:])

            # edge metadata, loaded once (shared by both layers)
            drel_sb = cp.tile([P, ECOLS], mybir.dt.int8)
            nc.sync.dma_start(drel_sb[:], drel_d[:])
            grel_sb = cp.tile([P, NW], F32)
            nc.sync.dma_start(grel_sb[:], grel_d[:])

            # persistent node buffers
            h_pre = pp.tile([P, NW, HC], F16)   # pre-BN node features
            se_sb = pp.tile([P, NW, 2 * H], F16)  # per-node es||ed (this layer)
            nc.gpsimd.memset(h_pre[:], 0.0)
            nc.gpsimd.memset(se_sb[:], 0.0)

            # BN1 column coefs (written after layer-1 stats AR)
            bn1_scale = pp.tile([P, 2], F32)
            bn1_shift = pp.tile([P, 2], F32)

            def rows_of(w):
                return LAST_ROWS if w == NW - 1 else P

            # ================= per-layer =================
            for l in range(2):
                # ---- prelude: GEMM h||res + es/ed, build tables, chunked AllGather
                for k in range(NCH):
                    for w in range(CH_WSTART[k], CH_WSTART[k] + CH_WINDOWS[k]):
                        rows = rows_of(w)
                        psum_h = pmm.tile([P, 2 * HC], F32, tag="psum_h")
                        psum_es = ped.tile([P, 2 * H], F32, tag="ped")
                        if l == 0:
                            lhs = wp.tile([P, P], F16, tag="xT")
                            nc.sync.dma_start(lhs[:, :rows], xcT[:, w * P : w * P + rows])
                            nc.tensor.matmul(
                                psum_h[:rows], lhsT=lhs[:, :rows], rhs=wcat1_s[:],
                                start=True, stop=True,
                            )
                            nc.tensor.matmul(
                                psum_es[:rows], lhsT=lhs[:, :rows], rhs=wse1_s[:],
                                start=True, stop=True,
                            )
                        else:
                            hin = h_pre[:, w, :]
                            for kk in range(2):
                                tr = ptr.tile([P, P], F16, tag="tr")
                                nc.tensor.transpose(
                                    tr[:, :rows], hin[:rows, kk * P : (kk + 1) * P],
                                    identity_h[:rows, :rows],
                                )
                                # fused BN1 apply + relu on the transposed tile
                                hT = wp.tile([P, P], F16, tag="xT")
                                nc.scalar.activation(
                                    hT[:, :rows], tr[:, :rows],
                                    mybir.ActivationFunctionType.Relu,
                                    bias=bn1_shift[:, kk : kk + 1],
                                    scale=bn1_scale[:, kk : kk + 1],
                                )
                                nc.tensor.matmul(
                                    psum_h[:rows],
                                    lhsT=hT[:, :rows],
                                    rhs=(wcat2_s0 if kk == 0 else wcat2_s1)[:],
                                    start=(kk == 0),
                                    stop=(kk == 1),
                                )
                                nc.tensor.matmul(
                                    psum_es[:rows],
                                    lhsT=hT[:, :rows],
                                    rhs=(wse2_s0 if kk == 0 else wse2_s1)[:],
                                    start=(kk == 0),
                                    stop=(kk == 1),
                                )
                        # residual + bias -> h_pre tile
                        nc.vector.tensor_add(
                            h_pre[:rows, w, :], psum_h[:rows, HC : 2 * HC], bias_s[l][:rows]
                        )
                        nc.scalar.copy(se_sb[:rows, w, :], psum_es[:rows])
                        # table row: h || es (fp16)
                        h_tb = wp.tile([P, TW], TBL_DT, tag="h_tb")
                        nc.scalar.copy(h_tb[:rows, 0:HC], psum_h[:rows, 0:HC])
                        nc.scalar.copy(h_tb[:rows, HC:TW], psum_es[:rows, 0:H])
                        ro = (w - CH_WSTART[k]) * P
                        nc.sync.dma_start(
                            tshard_c[l][k][ro : ro + rows, 0:TW], h_tb[:rows]
                        )
                    # fire this chunk's AllGather
                    nc.gpsimd.collective_compute(
                        "AllGather",
                        mybir.AluOpType.bypass,
                        replica_groups=RG,
                        ins=[tshard_c[l][k].ap()],
                        outs=[tfull[l][CH_GSTART[k] : CH_GSTART[k] + 8 * CH_ROWS[k], :]],
                    )

                if dbg and l == 0:
                    nc.gpsimd.dma_start(dbg_se[:], se_sb[:])
                    nc.gpsimd.dma_start(dbg_tfull[:], tfull[0][:])

                # ---- edge pass
                stats_acc = wp.tile([1, 2 * HC], F32, tag="stats_acc")
                nc.gpsimd.memset(stats_acc[:], 0.0)
                gtiles = {}

                def emit_gathers(w):
                    li = ep.tile([P, NLOW * 8], I16, tag="li", bufs=5)
                    nc.sync.dma_start(li[:], lowidx_d[:, w * NLOW * 8 : (w + 1) * NLOW * 8])
                    hi = ep.tile([P, NHIGH * 8], I16, tag="hi", bufs=5)
                    nc.sync.dma_start(hi[:], highidx_d[:, w * NHIGH * 8 : (w + 1) * NHIGH * 8])
                    ga = gbp.tile([P, NLOW, TWP], TBL_DT, tag="ga")
                    nc.gpsimd.dma_gather(
                        out_ap=ga[:],
                        in_ap=tfull[l][0:T_SPLIT, :],
                        idxs_ap=li[:],
                        num_idxs=NLOW * P,
                        num_idxs_reg=NLOW * P,
                        elem_size=TWP,
                        single_packet=False,
                        queue_num=2 * (w % 2),
                    )
                    gb = gbp.tile([P, NHIGH, TWP], TBL_DT, tag="gb")
                    nc.gpsimd.dma_gather(
                        out_ap=gb[:],
                        in_ap=tfull[l][HI_BASE:N, :],
                        idxs_ap=hi[:],
                        num_idxs=NHIGH * P,
                        num_idxs_reg=NHIGH * P,
                        elem_size=TWP,
                        single_packet=False,
                        queue_num=2 * (w % 2) + 1,
                    )
                    gtiles[w] = (ga, gb)

                for w in range(min(PF, NW)):
                    emit_gathers(w)

                for w in range(NW):
                    rows = rows_of(w)
                    ga, gb = gtiles.pop(w)
                    if w + PF < NW:
                        emit_gathers(w + PF)
                    # ind_T via int8 broadcast DMA (half the bytes of the f16 version)
                    drelT_rep = ep.tile([P, K_w * P], mybir.dt.int8, tag="drelT_rep", bufs=3)
                    nc.sync.dma_start(
                        drelT_rep[:], drelT_d[w : w + 1, :].to_broadcast([P, K_w * P])
                    )
                    ind_T = ep.tile([P, K_w, P], F16, tag="indT")
                    nc.vector.tensor_scalar(
                        out=ind_T[:],
                        in0=drelT_rep[:].rearrange("d (t e) -> d t e", e=P),
                        scalar1=iota_col[:, 0:1],
                        scalar2=None,
                        op0=mybir.AluOpType.is_equal,
                    )
                    # ed broadcast node->edge via transposed indicator matmuls
                    psum_ed = ped.tile([P, K_w * H], F32, tag="ped")
                    for t in range(K_w):
                        nc.tensor.matmul(
                            psum_ed[:, t * H : (t + 1) * H],
                            lhsT=ind_T[:, t, :],
                            rhs=se_sb[:, w, H : 2 * H],
                            start=True,
                            stop=True,
                        )
                    # s = es[src] + ed[dst]; scores on scalar engine
                    s_t = wp.tile([P, K_w, H], F16, tag="s_t")
                    nc.vector.tensor_tensor(
                        out=s_t[:, 0:NLOW, :],
                        in0=psum_ed[:].rearrange("p (t h) -> p t h", h=H)[:, 0:NLOW, :],
                        in1=ga[:, :, HC:TW],
                        op=mybir.AluOpType.add,
                    )
                    nc.vector.tensor_tensor(
                        out=s_t[:, NLOW:K_w, :],
                        in0=psum_ed[:].rearrange("p (t h) -> p t h", h=H)[:, NLOW:K_w, :],
                        in1=gb[:, :, HC:TW],
                        op=mybir.AluOpType.add,
                    )
                    lr_t = wp.tile([P, K_w, H], F16, tag="lr_t")
                    nc.vector.scalar_tensor_tensor(
                        out=lr_t[:], in0=s_t[:], scalar=SLOPE, in1=s_t[:],
                        op0=mybir.AluOpType.mult, op1=mybir.AluOpType.max,
                    )
                    ex_t = wp.tile([P, K_w, H], F16, tag="ex_t")
                    nc.scalar.activation(ex_t[:], lr_t[:], mybir.ActivationFunctionType.Exp)
                    if dbg and l == 0 and w == 0:
                        nc.gpsimd.dma_start(dbg_s[:], s_t[:])
                        nc.gpsimd.dma_start(dbg_ex[:], ex_t[:])
                    # indicator + staging per half
                    inds = []
                    stgs = []
                    for (tg, lo, nch), g_h in zip(
                        (("a", 0, NLOW), ("b", NLOW, NHIGH)), (ga, gb)
                    ):
                        sl = slice(lo, lo + nch)
                        ind_h = ep.tile([P, nch, P], F16, tag=f"ind_{tg}")
                        nc.vector.tensor_tensor(
                            out=ind_h[:],
                            in0=drel_sb[:, w * K_w + lo : w * K_w + lo + nch, None].to_broadcast(
                                [P, nch, P]
                            ),
                            in1=iota_h[:, None, :].to_broadcast([P, nch, P]),
                            op=mybir.AluOpType.is_equal,
                        )
                        inds.append(ind_h)
                        stg_h = ep.tile([P, nch, TW], F16, tag=f"stg_{tg}")
                        nc.vector.tensor_tensor(
                            out=stg_h[:, :, 0:HC].rearrange("p t (h c) -> p t h c", h=H),
                            in0=g_h[:, :, 0:HC].rearrange("p t (h c) -> p t h c", h=H),
                            in1=ex_t[:, sl, :, None].to_broadcast([P, nch, H, C]),
                            op=mybir.AluOpType.mult,
                        )
                        nc.scalar.copy(stg_h[:, :, HC:TW], ex_t[:, sl, :])
                        stgs.append(stg_h)
                    # aggregate
                    agg = pagg.tile([P, TW], F32, tag="agg")
                    for t in range(K_w):
                        half = 0 if t < NLOW else 1
                        tl = t if t < NLOW else t - NLOW
                        nc.tensor.matmul(
                            agg[:],
                            lhsT=inds[half][:, tl, :],
                            rhs=stgs[half][:, tl, :],
                            start=(t == 0),
                            stop=(t == K_w - 1),
                        )
                    if dbg and l == 0 and w == 0:
                        aggc = wp.tile([P, TW], F32, tag="aggc")
                        nc.scalar.copy(aggc[:], agg[:])
                        nc.sync.dma_start(dbg_agg[:], aggc[:])
                    # evict: h_pre += agg[:, :256] * rden
                    rden = wp.tile([P, H], F32, tag="rden")
                    nc.vector.reciprocal(rden[:rows], agg[:rows, HC:TW])
                    agn = wp.tile([P, HC], F32, tag="agn")
                    nc.vector.tensor_tensor(
                        out=agn[:rows].rearrange("p (h c) -> p h c", h=H),
                        in0=agg[:rows, 0:HC].rearrange("p (h c) -> p h c", h=H),
                        in1=rden[:rows, :, None].to_broadcast([rows, H, C]),
                        op=mybir.AluOpType.mult,
                    )
                    nc.vector.tensor_add(h_pre[:rows, w, :], agn[:rows], h_pre[:rows, w, :])
                    # BN stats: per-window matmuls + SBUF accumulate
                    hsq = wp.tile([P, HC], F16, tag="hsq")
                    nc.scalar.square(hsq[:rows], h_pre[:rows, w, :])
                    st = ptr.tile([1, 2 * HC], F32, tag="tr")
                    nc.tensor.matmul(
                        st[0:1, 0:HC], lhsT=ones_col[:rows, :], rhs=h_pre[:rows, w, :],
                        start=True, stop=True,
                    )
                    nc.tensor.matmul(
                        st[0:1, HC : 2 * HC], lhsT=ones_col[:rows, :], rhs=hsq[:rows],
                        start=True, stop=True,
                    )
                    nc.vector.tensor_add(stats_acc[:], stats_acc[:], st[:])

                # ---- BN stats all-reduce
                nc.sync.dma_start(stats_in[l].ap().rearrange("(a b) c -> b (a c)", b=1), stats_acc[:])
                nc.gpsimd.collective_compute(
                    "AllReduce",
                    mybir.AluOpType.add,
                    replica_groups=RG,
                    ins=[stats_in[l].ap()],
                    outs=[stats_out[l].ap()],
                )
                if l == 0:
                    # column-form coefs for the fused layer-2 prelude BN
                    sums = wp.tile([P, 2], F32, tag="sums")
                    sqs = wp.tile([P, 2], F32, tag="sqs")
                    for kk in range(2):
                        nc.sync.dma_start(
                            sums[:, kk : kk + 1], stats_out[0][kk * P : (kk + 1) * P, :]
                        )
                        nc.sync.dma_start(
                            sqs[:, kk : kk + 1], stats_out[0][HC + kk * P : HC + (kk + 1) * P, :]
                        )
                    mean_c = wp.tile([P, 2], F32, tag="mean_c")
                    nc.vector.tensor_scalar_mul(mean_c[:], sums[:], 1.0 / N)
                    var_c = wp.tile([P, 2], F32, tag="var_c")
                    nc.vector.tensor_scalar_mul(var_c[:], sqs[:], 1.0 / N)
                    msq_c = wp.tile([P, 2], F32, tag="msq_c")
                    nc.vector.tensor_tensor(out=msq_c[:], in0=mean_c[:], in1=mean_c[:], op=mybir.AluOpType.mult)
                    nc.vector.tensor_tensor(out=var_c[:], in0=var_c[:], in1=msq_c[:], op=mybir.AluOpType.subtract)
                    nc.vector.tensor_scalar_add(var_c[:], var_c[:], EPS)
                    sq_c = wp.tile([P, 2], F32, tag="sq_c")
                    nc.scalar.activation(sq_c[:], var_c[:], mybir.ActivationFunctionType.Sqrt)
                    inv_c = wp.tile([P, 2], F32, tag="inv_c")
                    nc.vector.reciprocal(inv_c[:], sq_c[:])
                    nc.vector.tensor_tensor(out=bn1_scale[:], in0=inv_c[:], in1=bngc_s[:], op=mybir.AluOpType.mult)
                    sh_c = wp.tile([P, 2], F32, tag="sh_c")
                    nc.vector.tensor_tensor(out=sh_c[:], in0=mean_c[:], in1=bn1_scale[:], op=mybir.AluOpType.mult)
                    nc.vector.tensor_tensor(out=bn1_shift[:], in0=bnbc_s[:], in1=sh_c[:], op=mybir.AluOpType.subtract)
                    if dbg:
                        nc.gpsimd.dma_start(dbg_hpre[:], h_pre[:])
                        nc.sync.dma_start(dbg_stats[:], stats_out[0][:])
                        nc.sync.dma_start(dbg_bn[:, 0:2], bn1_scale[:])
                        nc.sync.dma_start(dbg_bn[:, 2:4], bn1_shift[:])
                else:
                    if dbg:
                        nc.gpsimd.dma_start(dbg_hpre2[:], h_pre[:])
                        nc.gpsimd.dma_start(dbg_se2[:], se_sb[:])
                    # row-form BN2 apply + relu on h_pre
                    stats_g = wp.tile([1, 2 * HC], F32, tag="stats_g")
                    nc.sync.dma_start(
                        stats_g[:], stats_out[1].ap().rearrange("(a b) c -> b (a c)", b=1)
                    )
                    bngs = wp.tile([1, HC], F32, tag="bngs")
                    nc.sync.dma_start(bngs[:], bng2[:])
                    bnbs = wp.tile([1, HC], F32, tag="bnbs")
                    nc.sync.dma_start(bnbs[:], bnb2[:])
                    mrow = wp.tile([1, HC], F32, tag="mrow")
                    nc.vector.tensor_scalar_mul(mrow[:], stats_g[0:1, 0:HC], 1.0 / N)
                    vrow = wp.tile([1, HC], F32, tag="vrow")
                    nc.vector.tensor_scalar_mul(vrow[:], stats_g[0:1, HC : 2 * HC], 1.0 / N)
                    msq = wp.tile([1, HC], F32, tag="msq")
                    nc.vector.tensor_tensor(out=msq[:], in0=mrow[:], in1=mrow[:], op=mybir.AluOpType.mult)
                    nc.vector.tensor_tensor(out=vrow[:], in0=vrow[:], in1=msq[:], op=mybir.AluOpType.subtract)
                    nc.vector.tensor_scalar_add(vrow[:], vrow[:], EPS)
                    sq = wp.tile([1, HC], F32, tag="sq")
                    nc.scalar.activation(sq[:], vrow[:], mybir.ActivationFunctionType.Sqrt)
                    inv = wp.tile([1, HC], F32, tag="inv")
                    nc.vector.reciprocal(inv[:], sq[:])
                    scale = wp.tile([1, HC], F32, tag="scale")
                    nc.vector.tensor_tensor(out=scale[:], in0=inv[:], in1=bngs[:], op=mybir.AluOpType.mult)
                    shift = wp.tile([1, HC], F32, tag="shift")
                    nc.vector.tensor_tensor(out=shift[:], in0=mrow[:], in1=scale[:], op=mybir.AluOpType.mult)
                    nc.vector.tensor_tensor(out=shift[:], in0=bnbs[:], in1=shift[:], op=mybir.AluOpType.subtract)
                    scale_rep = wp.tile([P, HC], F32, tag="scale_rep")
                    nc.gpsimd.partition_broadcast(scale_rep[:], scale[:])
                    shift_rep = wp.tile([P, HC], F32, tag="shift_rep")
                    nc.gpsimd.partition_broadcast(shift_rep[:], shift[:])
                    if dbg:
                        nc.sync.dma_start(dbg_stats2[:], stats_out[1][:])
                        nc.sync.dma_start(dbg_scale2[:, 0:HC], scale_rep[:])
                        nc.sync.dma_start(dbg_scale2[:, HC:2*HC], shift_rep[:])
                    for w in range(NW):
                        rows = rows_of(w)
                        nc.vector.tensor_tensor(
                            out=h_pre[:rows, w, :], in0=h_pre[:rows, w, :], in1=scale_rep[:rows],
                            op=mybir.AluOpType.mult,
                        )
                        nc.vector.tensor_tensor(
                            out=h_pre[:rows, w, :], in0=h_pre[:rows, w, :], in1=shift_rep[:rows],
                            op=mybir.AluOpType.add,
                        )
                        nc.scalar.activation(
                            h_pre[:rows, w, :], h_pre[:rows, w, :], mybir.ActivationFunctionType.Relu
                        )

            if dbg:
                nc.gpsimd.dma_start(dbg_hbn2[:], h_pre[:])
            # ================= pooling =================
            pool_ps = pmm.tile([P, HC], F32, tag="psum_h")
            for w in range(NW):
                rows = rows_of(w)
                gind = wp.tile([P, P], F16, tag="gind")
                nc.vector.tensor_tensor(
                    out=gind[:],
                    in0=grel_sb[:, w : w + 1].to_broadcast([P, P]),
                    in1=iota_f[:],
                    op=mybir.AluOpType.is_equal,
                )
                nc.tensor.matmul(
                    pool_ps[:],
                    lhsT=gind[:rows, :],
                    rhs=h_pre[:rows, w, :],
                    start=(w == 0),
                    stop=(w == NW - 1),
                )
            pooled_sb = wp.tile([P, HC], F32, tag="pooled_sb")
            nc.scalar.copy(pooled_sb[:], pool_ps[:])
            zt = wp.tile([P, HC], F32, tag="zt")
            nc.gpsimd.memset(zt[:], 0.0)
            for r in range(0, G, P):
                nc.sync.dma_start(pooled_in[r : r + P, :], zt[:])
            nc.sync.dma_start(pooled_in[G : G + 1, :], zt[0:1, :])
            scat = wp.tile([P, 1], I32, tag="scat")
            nc.sync.dma_start(scat[:], scatidx_d[:])
            nc.gpsimd.indirect_dma_start(
                out=pooled_in[:],
                out_offset=bass.IndirectOffsetOnAxis(ap=scat[:, :1], axis=0),
                in_=pooled_sb[:],
                in_offset=None,
            )
            nc.gpsimd.collective_compute(
                "AllReduce",
                mybir.AluOpType.add,
                replica_groups=RG,
                ins=[pooled_in[0:G, :]],
                outs=[pooled_out.ap()],
            )

            if dbg:
                nc.sync.dma_start(dbg_pool[:], pooled_out[:])
            # ================= MLP =================
            fc1w_s0 = cp.tile([P, 128], F32)
            nc.sync.dma_start(fc1w_s0[:], fc1w[0:P, :])
            fc1w_s1 = cp.tile([P, 128], F32)
            nc.sync.dma_start(fc1w_s1[:], fc1w[P:HC, :])
            fc2w_s = cp.tile([P, 2], F32)
            nc.sync.dma_start(fc2w_s[:], fc2w[:])
            fc1b_s = cp.tile([P, 128], F32)
            nc.sync.dma_start(fc1b_s[:], fc1b[:])
            fc2b_s = cp.tile([P, 2], F32)
            nc.sync.dma_start(fc2b_s[:], fc2b[:])

            for tgi in range(G // P):
                pt = wp.tile([P, HC], F32, tag="pt")
                nc.sync.dma_start(pt[:], pooled_out[tgi * P : (tgi + 1) * P, :])
                rc = wp.tile([P, 1], F32, tag="rc")
                nc.sync.dma_start(rc[:], rcnt[tgi * P : (tgi + 1) * P, :])
                nc.vector.tensor_scalar_mul(pt[:], pt[:], rc[:, 0:1])
                mm1 = pmm.tile([P, 128], F32, tag="psum_h")
                for k in range(2):
                    tr = ptr.tile([P, P], F32, tag="tr")
                    nc.tensor.transpose(tr[:], pt[:, k * P : (k + 1) * P], identity[:])
                    ptT = wp.tile([P, P], F32, tag="ptT")
                    nc.scalar.copy(ptT[:], tr[:])
                    nc.tensor.matmul(
                        mm1[:],
                        lhsT=ptT[:],
                        rhs=(fc1w_s0 if k == 0 else fc1w_s1)[:],
                        start=(k == 0),
                        stop=(k == 1),
                    )
                o1 = wp.tile([P, 128], F32, tag="o1")
                nc.vector.tensor_add(o1[:], mm1[:], fc1b_s[:])
                nc.scalar.activation(o1[:], o1[:], mybir.ActivationFunctionType.Relu)
                tr2 = ptr.tile([P, P], F32, tag="tr")
                nc.tensor.transpose(tr2[:], o1[:], identity[:])
                o1T = wp.tile([P, P], F32, tag="o1T")
                nc.scalar.copy(o1T[:], tr2[:])
                mm2 = pagg.tile([P, 2], F32, tag="agg")
                nc.tensor.matmul(mm2[:], lhsT=o1T[:], rhs=fc2w_s[:], start=True, stop=True)
                oo = wp.tile([P, 2], F32, tag="oo")
                nc.vector.tensor_add(oo[:], mm2[:], fc2b_s[:])
                nc.sync.dma_start(out_d[tgi * P : (tgi + 1) * P, :], oo[:])

    nc.compile()
    return nc


_PROGRAM_CACHE: dict[tuple, object] = {}


def _get_program(split: tuple):
    if split not in _PROGRAM_CACHE:
        _PROGRAM_CACHE[split] = _build_program(split)
    return _PROGRAM_CACHE[split]


# ---------------------------------------------------------------- entry point

def kernel(
    x, edge_index, edge_attr, batch,
    W1, att_src1, att_dst1, b1, res1_W, res1_b, bn1_g, bn1_b,
    W2, att_src2, att_dst2, b2, res2_W, res2_b, bn2_g, bn2_b,
    fc1_W, fc1_b, fc2_W, fc2_b,
    _run_opts: dict | None = None,
):
    x = np.asarray(x, np.float32)
    edge_index = np.asarray(edge_index)
    batch = np.asarray(batch)

    metas, split = _host_meta(edge_index, batch)
    nc = _get_program(split)

    W1n = np.asarray(W1, np.float32)
    W2n = np.asarray(W2, np.float32)
    wcat1 = np.ascontiguousarray(np.concatenate([W1n, np.asarray(res1_W)], axis=1)).astype(np.float16)
    wcat2 = np.ascontiguousarray(np.concatenate([W2n, np.asarray(res2_W)], axis=1)).astype(np.float16)
    wse1 = np.ascontiguousarray(
        np.concatenate([W1n @ _att_mat(att_src1), W1n @ _att_mat(att_dst1)], axis=1)
    ).astype(np.float16)
    wse2 = np.ascontiguousarray(
        np.concatenate([W2n @ _att_mat(att_src2), W2n @ _att_mat(att_dst2)], axis=1)
    ).astype(np.float16)
    rep = lambda v: np.ascontiguousarray(np.tile(np.asarray(v, np.float32).reshape(1, -1), (P, 1)))
    cnt = np.bincount(batch, minlength=G).astype(np.float32)
    rcnt = (1.0 / np.maximum(cnt, 1.0)).astype(np.float32)[:, None]

    shared = dict(
        wcat1=wcat1, wse1=wse1, wcat2=wcat2, wse2=wse2,
        bias1=rep(np.asarray(b1) + np.asarray(res1_b)), bias2=rep(np.asarray(b2) + np.asarray(res2_b)),
        bngc=np.ascontiguousarray(np.asarray(bn1_g, np.float32).reshape(2, P).T),
        bnbc=np.ascontiguousarray(np.asarray(bn1_b, np.float32).reshape(2, P).T),
        bng2=np.asarray(bn2_g, np.float32).reshape(1, -1), bnb2=np.asarray(bn2_b, np.float32).reshape(1, -1),
        fc1w=np.asarray(fc1_W, np.float32), fc2w=np.asarray(fc2_W, np.float32),
        fc1b=rep(fc1_b), fc2b=rep(fc2_b),
        rcnt=rcnt,
    )
    in_maps = []
    for c in range(M):
        m = metas[c]
        in_maps.append(
            dict(
                shared,
                xcT=np.ascontiguousarray(x[c * NCN : (c + 1) * NCN].T).astype(np.float16),
                grel=m["grel"], scatidx=m["scatidx"],
                lowidx=m["lowidx"], highidx=m["highidx"], drel=m["drel"], drelT=m["drelT"],
            )
        )

    opts = _run_opts or {}
    res = run_bass_kernel_spmd(nc, in_maps, core_ids=list(range(M)), **opts)
    out = res.results[0]["out"].astype(np.float32)
    if opts:
        kernel.last_result = res  # stash for profiling harnesses
    return out
